# revision 22
# baseline (speedup 1.0000x reference)
"""Trainium2 Bass kernel for nn_MultiHeadAttention_80418967650946.

Reference computation (per batch b):
  qp/kp/vp = 1x1-conv projections of q/k/v   [64, N]
  funky head view: qh[h,n,d] = qp.reshape(4, 16*N)[d, 16n+h]  (same for kh, vh)
  scores = qh @ kh * 0.25^0.5 + bias ; attn = softmax(scores)
  x[4h+d, n] = (attn @ vh)[h, n, d] ; y = LeakyReLU(BN(Wo @ x + bo), 0.2)

Sharding: 8 cores = 4 batches x 2 query-halves (n in [0,512) or [512,1024)).
Each core computes its query-half for ALL 16 heads fully locally (no
collectives): the output conv is column-wise independent, so y[:, n-half]
only needs x[:, n-half].

Per-core device algorithm (all fp32):
  - projections on TensorE produce Kp2 [4, 16384] (d-major, J=16m+h free),
    Qp2 [4, 8192] (pre-scaled by 0.25^0.5), and Vt [128, 1280] where
    Vt[r, 80s+c] = VpT[16r+s, c] for c<64 and 1.0 for c in [64,80)
    (rows 64:128 duplicate 0:64 for the second K=64 row-group).
  - scoresT[m-chunk, n] psum tiles come from one K=4 matmul; the additive
    bias is injected into the SAME psum by an identity matmul over the
    (host-pre-transposed) bias tile; exp() runs on ScalarE psum->sbuf.
  - attn@V contracts m on partitions via two K=64 row-tiled matmuls whose
    lhsT carries a ones column -> softmax denominators come for free.
  - per-head normalization assembles x into a PSUM tile (PSUM APs have no
    32-partition base alignment restriction), then one copy -> SBUF feeds
    the output matmul + fused BN-affine + LeakyReLU epilogue.
"""
import sys

if "/opt/trn_rl_repo" not in sys.path:
    sys.path.insert(0, "/opt/trn_rl_repo")

import numpy as np

import concourse.bass as bass
import concourse.tile as tile
from concourse import bacc, mybir
from concourse.bass_utils import run_bass_kernel_spmd

F32 = mybir.dt.float32
AF = mybir.ActivationFunctionType
ALU = mybir.AluOpType
PSUM = bass.MemorySpace.PSUM
F32R = mybir.dt.float32r
BF16 = mybir.dt.bfloat16




H = 16
D = 4
HID = 256
B = 4
N = 1024
NH = 512          # per-core query positions
NCORES = 8
SCALE = float(D) ** -0.5
BN_EPS = 1e-5
NEG_SLOPE = 0.2


def _emit(nc, tc, io):
    kb, qb, vb = io["kb"], io["qb"], io["vb"]
    biasT, wkT, wvT, wqT, woT = io["biasT"], io["wkT"], io["wvT"], io["wqT"], io["woT"]
    bnv, y = io["bnv"], io["y"]

    with (
        tc.tile_pool(name="persist", bufs=1) as persist,
        tc.tile_pool(name="bias", bufs=2) as bp,
        tc.tile_pool(name="exp", bufs=16) as ep,
        tc.tile_pool(name="sml", bufs=2) as sp,
        tc.tile_pool(name="p1", bufs=1) as p1,
        tc.tile_pool(name="ps_s", bufs=3, space=PSUM) as pss,
        tc.tile_pool(name="ps_x", bufs=2, space=PSUM) as psx,
    ):
        Kp2 = persist.tile([100, H * N], BF16, tag="Kp2")
        Qp2 = persist.tile([100, H * NH], BF16, tag="Qp2")
        Vtm = persist.tile([128, H * 8 * 5], BF16, tag="Vtm")
        x_sb = persist.tile([64, NH], F32R, tag="x_sb")
        woT_sb = persist.tile([64, HID], F32R, tag="woT_sb")
        s_sb = persist.tile([128, 2], F32, tag="s_sb")
        t_sb = persist.tile([128, 2], F32, tag="t_sb")

        # ---- bias prefetch: no dependencies, issue at t=0 ----
        # host layout [H, 128, 8, 512] = (h, p, t, n): one contiguous 16 KiB
        # read per (partition, head).
        bias_tiles = {}
        for h0 in (0, 2):
            bh2 = bp.tile([128, 8192], F32, tag="bh2")
            nc.sync.dma_start(
                bh2[:].rearrange("p (h t n) -> p h t n", h=2, t=8),
                biasT[h0:h0 + 2].rearrange("h p t n -> p h t n"))
            bias_tiles[h0] = bh2

        # ---------------- phase 1: projections + BN vectors ----------------
        k_sb = p1.tile([128, 2048], BF16, tag="k_sb")
        q_sb = p1.tile([128, 2048], BF16, tag="q_sb")
        v_sb = p1.tile([128, 2048], BF16, tag="v_sb")
        nc.gpsimd.dma_start(q_sb[:].rearrange("p (c n) -> p c n", c=2),
                            qb.rearrange("(c p) n -> p c n", p=128))
        nc.gpsimd.dma_start(k_sb[:].rearrange("p (c n) -> p c n", c=2),
                            kb.rearrange("(c p) n -> p c n", p=128))
        nc.gpsimd.dma_start(v_sb[:].rearrange("p (c n) -> p c n", c=2),
                            vb.rearrange("(c p) n -> p c n", p=128))
        wk_sb = p1.tile([128, 128], BF16, tag="wk_sb")
        wv_sb = p1.tile([128, 128], BF16, tag="wv_sb")
        wq_sb = p1.tile([128, 64], BF16, tag="wq_sb")
        nc.gpsimd.dma_start(wq_sb[:].rearrange("p (c o) -> p c o", c=2),
                            wqT.rearrange("(c p) o -> p c o", p=128))
        nc.gpsimd.dma_start(wk_sb[:].rearrange("p (c o) -> p c o", c=2),
                            wkT.rearrange("(c p) o -> p c o", p=128))
        nc.gpsimd.dma_start(wv_sb[:].rearrange("p (c o) -> p c o", c=2),
                            wvT.rearrange("(c p) o -> p c o", p=128))
        nc.gpsimd.dma_start(woT_sb[:], woT)

        # BN affine: s = gamma * rsqrt(var+eps), t = (bo - mean) * s + beta
        bn_sb = p1.tile([128, 10], F32, tag="bn_sb")
        nc.gpsimd.dma_start(bn_sb[:], bnv)
        tmp = p1.tile([128, 2], F32, tag="tmp")
        tmp2 = p1.tile([128, 2], F32, tag="tmp2")
        nc.vector.tensor_scalar_add(tmp[:], bn_sb[:, 6:8], BN_EPS)
        nc.scalar.sqrt(tmp[:], tmp[:])
        nc.vector.reciprocal(tmp[:], tmp[:])
        nc.vector.tensor_mul(s_sb[:], bn_sb[:, 0:2], tmp[:])
        nc.vector.tensor_sub(tmp2[:], bn_sb[:, 8:10], bn_sb[:, 4:6])
        nc.vector.tensor_mul(tmp2[:], tmp2[:], s_sb[:])
        nc.vector.tensor_add(t_sb[:], tmp2[:], bn_sb[:, 2:4])

        # Q/K projections, 4 j-values col-tiled per [128,1024] psum tile
        # (rows 32g+d hold j = 4*b4+g), then one wide ACT copy -> staging and a
        # partition-scatter DMA into the 4-partition Kp2/Qp2 layout.
        def proj(w_sb, x_in, dst, nj, b4, scale):
            psp = pss.tile([128, 1024], F32, tag="ps")
            nc.vector.memset(psp[:], 0.0)
            for g in range(4):
                j = 4 * b4 + g
                for nn2 in range(2):
                    for c in range(2):
                        nc.tensor.matmul(
                            psp[32 * g:32 * g + 4, 512 * nn2:512 * nn2 + 512],
                            w_sb[:, nj * c + _pj(nj, j):nj * c + _pj(nj, j) + _pw(nj):_ps(nj)],
                            x_in[:, 1024 * c + 512 * nn2:1024 * c + 512 * nn2 + 512],
                            start=(c == 0), stop=(c == 1), tile_position=(0, 32 * g))
            stg = sp.tile([128, 1024], BF16, tag="stg")
            nc.scalar.mul(stg[:], psp[:], scale)
            for g in range(4):
                nc.gpsimd.dma_start(
                    dst[0:4, 4096 * b4 + 1024 * g:4096 * b4 + 1024 * g + 1024],
                    stg[32 * g:32 * g + 4, :])

        for b4 in range(2):
            proj(wq_sb, q_sb, Qp2, 32, b4, SCALE)
        for b4 in range(4):
            proj(wk_sb, k_sb, Kp2, 64, b4, 1.0)

        # V projection into Vtm [128, (h, t, c5)] bf16:
        #   Vtm[p, 40h + 5t + 0]     = 1.0   (ones column -> softmax denom)
        #   Vtm[p, 40h + 5t + 1 + d] = vh[m = 128t + p, d]  for head h
        for s in range(16):
            psv = psx.tile([64, 64], F32, tag="ps5")
            for c in range(2):
                nc.tensor.matmul(
                    psv[:],
                    v_sb[:, 1024 * c + s:1024 * c + s + 1009:16],
                    wv_sb[:, 64 * c:64 * c + 64],
                    start=(c == 0), stop=(c == 1),
                )
            pv = psv[:].rearrange("r (d c2) -> r d c2", c2=16)
            dst = Vtm[:].rearrange("p (h t c) -> p h t c", t=8, c=5)
            nc.vector.tensor_copy(dst[0:64, s, :, 1:5],
                                  pv[:, :, 0:16:2].transpose([0, 2, 1]))
            nc.vector.tensor_copy(dst[64:128, s, :, 1:5],
                                  pv[:, :, 1:16:2].transpose([0, 2, 1]))
        ones_f32 = p1.tile([128, 128], F32, tag="ones_f32")
        nc.vector.memset(ones_f32[:], 1.0)
        nc.vector.tensor_copy(
            Vtm[:].rearrange("p (h t c) -> p h t c", t=8, c=5)[:, :, :, 0],
            ones_f32[:].rearrange("p (h t) -> p h t", t=8))
        for rep in range(1, 4):
            nc.gpsimd.dma_start(Kp2[32 * rep:32 * rep + 4, :], Kp2[0:4, :])
            nc.gpsimd.dma_start(Qp2[32 * rep:32 * rep + 4, :], Qp2[0:4, :])

        # ---------------- phase 2: attention ----------------
        Kv = [Kp2[32 * rg:32 * rg + 4, :].rearrange("d (m s) -> d m s", s=16)
              for rg in range(4)]
        Qv = [Qp2[32 * rg:32 * rg + 4, :].rearrange("d (n s) -> d n s", s=16)
              for rg in range(4)]
        for h in range(H):
            if h % 2 == 0:
                if h in bias_tiles:
                    bh2 = bias_tiles[h]
                else:
                    bh2 = bp.tile([128, 8192], F32, tag="bh2")
                    nc.sync.dma_start(
                        bh2[:].rearrange("p (h t n) -> p h t n", h=2, t=8),
                        biasT[h:h + 2].rearrange("h p t n -> p h t n"))
            hb = 4096 * (h % 2)
            exps = []
            for u in range(4):   # pairs of m-chunks -> one 2-bank psum tile
                ps = pss.tile([128, 1024], F32, tag="ps")
                for v2 in range(2):
                    t = 2 * u + v2
                    rg = t % 4
                    nc.tensor.matmul(ps[:, 512 * v2:512 * v2 + 512],
                                     Kv[rg][:, 128 * t:128 * t + 128, h],
                                     Qv[rg][:, :, h],
                                     start=True, stop=True,
                                     tile_position=(32 * rg, 0))
                nc.vector.tensor_add(ps[:], ps[:], bh2[:, hb + 1024 * u:hb + 1024 * u + 1024])
                ex = ep.tile([128, 1024], BF16, tag="ex")
                nc.scalar.activation(ex[:], ps[:], AF.Exp)
                exps.append(ex)
            # attn@V: one K=128 matmul per m-chunk; lhsT column 0 is the ones
            # column -> psum row 0 = softmax denominator, rows 1..5 = x
            ps5 = psx.tile([5, NH], F32, tag="ps5")
            for t in range(8):
                nc.tensor.matmul(
                    ps5[:],
                    Vtm[:, 40 * h + 5 * t:40 * h + 5 * t + 5],
                    exps[t // 2][:, 512 * (t % 2):512 * (t % 2) + 512],
                    start=(t == 0), stop=(t == 7))
            d5 = sp.tile([5, NH], F32, tag="d5")
            nc.scalar.copy(d5[:], ps5[:])
            r5p = sp.tile([5, NH], F32, tag="r5p")
            nc.gpsimd.partition_broadcast(r5p[:], d5[0:1, :])
            r5 = sp.tile([5, NH], F32, tag="r5")
            nc.vector.reciprocal_approx_fast(r5[:], r5p[:])
            m5 = sp.tile([5, NH], F32R, tag="m5")
            nc.vector.tensor_mul(m5[:], d5[:], r5[:])
            nc.gpsimd.dma_start(x_sb[4 * h:4 * h + 4, :], m5[1:5, :])

        # ---------------- phase 3: output conv + BN + LeakyReLU ----------------
        for u in range(2):
            psy = pss.tile([128, NH], F32, tag="ps")
            nc.tensor.matmul(psy[:], woT_sb[0:64, 128 * u:128 * u + 128], x_sb[:],
                             start=True, stop=True)
            y2 = sp.tile([128, NH], F32, tag="y2")
            nc.vector.tensor_scalar(y2[:], psy[:], s_sb[:, u:u + 1], t_sb[:, u:u + 1],
                                    ALU.mult, ALU.add)
            yt = sp.tile([128, NH], F32, tag="yt")
            nc.vector.scalar_tensor_tensor(yt[:], y2[:], NEG_SLOPE, y2[:],
                                           ALU.mult, ALU.max)
            nc.sync.dma_start(y[128 * u:128 * u + 128, :], yt[:])


def _pj(nj, j):
    # column start of j's d-group in the weight tile: wq is (j,d)-packed with
    # 4 consecutive cols per j; wk is natural order o = 16d + j
    return 4 * j if nj == 32 else j


def _pw(nj):
    return 4 if nj == 32 else 49


def _ps(nj):
    return 1 if nj == 32 else 16


def build_program():
    nc = bacc.Bacc("TRN2", target_bir_lowering=False, debug=False)
    io = {
        "kb": nc.dram_tensor("kb", [HID, N], F32, kind="ExternalInput").ap(),
        "qb": nc.dram_tensor("qb", [HID, N], F32, kind="ExternalInput").ap(),
        "vb": nc.dram_tensor("vb", [HID, N], F32, kind="ExternalInput").ap(),
        "biasT": nc.dram_tensor("biasT", [H, 128, 8, NH], F32, kind="ExternalInput").ap(),
        "wkT": nc.dram_tensor("wkT", [HID, 64], F32, kind="ExternalInput").ap(),
        "wvT": nc.dram_tensor("wvT", [HID, 64], F32, kind="ExternalInput").ap(),
        "wqT": nc.dram_tensor("wqT", [HID, 32], F32, kind="ExternalInput").ap(),
        "woT": nc.dram_tensor("woT", [64, HID], F32, kind="ExternalInput").ap(),
        "bnv": nc.dram_tensor("bnv", [128, 10], F32, kind="ExternalInput").ap(),
        "y": nc.dram_tensor("y", [HID, NH], F32, kind="ExternalOutput").ap(),
    }
    with tile.TileContext(nc) as tc:
        _emit(nc, tc, io)
    nc.compile()
    return nc


def make_in_maps(q, k, v, attn_bias, Wq, Wk, Wv, Wo, bo, gamma, beta, run_mean, run_var):
    def f32(x):
        return np.ascontiguousarray(np.asarray(x, dtype=np.float32))

    q, k, v, attn_bias = f32(q), f32(k), f32(v), f32(attn_bias)
    Wq, Wk, Wv, Wo, bo = f32(Wq), f32(Wk), f32(Wv), f32(Wo), f32(bo)
    gamma, beta, run_mean, run_var = f32(gamma), f32(beta), f32(run_mean), f32(run_var)

    wkT = f32(Wk.T)
    wvT = f32(Wv.T)
    woT = f32(Wo.T)
    bnv = np.concatenate(
        [x.reshape(2, 128).T for x in (gamma, beta, run_mean, run_var, bo)], axis=1
    )
    bnv = f32(bnv)

    in_maps = []
    for core in range(NCORES):
        b, half = divmod(core, 2)
        n0 = half * NH
        rows = np.array([16 * d + 8 * half + jl for jl in range(8) for d in range(4)])
        wqT = f32(Wq[rows, :].T)                                  # [256, 32], col = 4*jl+d
        bt = attn_bias[b, :, n0:n0 + NH, :].transpose(0, 2, 1)          # [16, 1024m, 512n]
        biasT = f32(bt.reshape(H, 8, 128, NH).transpose(0, 2, 1, 3))    # [16, 128p, 8t, 512n]
        in_maps.append({
            "kb": f32(k[b]), "qb": f32(q[b]), "vb": f32(v[b]),
            "biasT": biasT, "wkT": wkT, "wvT": wvT, "wqT": wqT, "woT": woT,
            "bnv": bnv,
        })
    return in_maps


_NC_CACHE = None


def get_nc():
    global _NC_CACHE
    if _NC_CACHE is None:
        _NC_CACHE = build_program()
    return _NC_CACHE


def kernel(**inputs):
    nc = get_nc()
    in_maps = make_in_maps(**inputs)
    res = run_bass_kernel_spmd(nc, in_maps, list(range(NCORES)))
    out = np.empty((B, HID, N), dtype=np.float32)
    for core in range(NCORES):
        b, half = divmod(core, 2)
        out[b, :, half * NH:(half + 1) * NH] = res.results[core]["y"]
    return out
